# revision 16
# baseline (speedup 1.0000x reference)
"""Trainium2 Bass kernel for the 2-layer liquid-NN multistep recurrence.

Math (reference):
    for t in 0..49:
        h0 = 0.9*h0 + 0.1*tanh(h0 @ Wh0 + x_t @ Wu0 + b0)
        h1 = 0.9*h1 + 0.1*tanh(h1 @ Wh1 + h0 @ Wu1 + b1)
    out = h1 @ fc_w + fc_b

Kernel strategy:
  - Data parallel over 8 NeuronCores: batch 8192 -> 1024 rows/core.
  - State kept TRANSPOSED in SBUF: tiles are [128(h), 512(b)], matmuls
    contract over the partition dim with naturally-laid-out weights.
  - Rescaled state g_t = h_t / 0.9^t turns the update into a single fused
    axpy per tile:  g += (0.1*0.9^-(t+1)) * tanh(s*psum + b).  Wu1 is
    pre-scaled by 0.9 host-side so both cell-1 terms share the 0.9^t scale.
  - The H x H matmuls run as fp8e4m3 DoubleRow (K=256/instruction, 2x PE
    rate).  Weight error is suppressed by a hi+lo split (two fp8 matmuls
    at one shared power-of-2 scale = effective ~bf16 weights); the fp16
    master state is quantized to fp8 only at the matmul input each step,
    so quantization error does not compound in the state.  Measured
    end-to-end rel err ~6.7e-3 (gate 2e-2).
  - Weights are pre-scaled by 2^11 so fp8 stays in its normal range; the
    fp16 x-projection weights carry the same scale and the tanh's ACT
    scale divides it back out -- all for free on existing instructions.
  - x ships compact as [8t+f, b] fp16 (rows past 8T zero-padded host-side),
    DMA'd as four dense [128, b] tiles = 16 timesteps each.  The x_t @ Wu0
    term is a full K=128 fp16 matmul against one of 16 on-device-built
    weight tiles (Wu0 at rows 8j..8j+8, zeros elsewhere).
  - t=0 is specialized (state starts at zero): the Wh*state matmuls are
    skipped and the state is written directly, so no state memsets.
"""

import os
import sys

import numpy as np

for _p in ("/opt/trn_rl_repo",):
    if _p not in sys.path:
        sys.path.insert(0, _p)

import concourse.bass as bass
import concourse.tile as tile
from concourse import bacc, bass_utils, mybir

F32 = mybir.dt.float32
F16 = mybir.dt.float16
F8 = mybir.dt.float8e4
AF = mybir.ActivationFunctionType
ALU = mybir.AluOpType
DR = mybir.MatmulPerfMode.DoubleRow

NCORES = 8
B = 8192
BL = B // NCORES  # 1024
S = 50
F = 8
H = 512
P = 10
T = 50
DT = 0.1
DEC = 1.0 - DT
KT = H // 128  # 4 k tiles
NP_ = KT // 2  # 2 DoubleRow k-pairs
NH = 2  # batch halves of 512
NHW = BL // NH  # 512
WS = 2048.0  # fp8 weight pre-scale (2^11)

TPX = 128 // F  # 16 timesteps per x tile
NXT = (T + TPX - 1) // TPX  # 4 x tiles
XROWS = NXT * 128


def build_program():
    nc = bacc.Bacc(
        "TRN2", target_bir_lowering=False, debug=False, num_devices=NCORES
    )
    xT_d = nc.dram_tensor("xTp", [XROWS, BL], F16, kind="ExternalInput").ap()
    w_d = {}
    for nm in ("Wh0h", "Wh0l", "Wh1h", "Wh1l", "Wu1h", "Wu1l"):
        w_d[nm] = nc.dram_tensor(nm, [H, H], F8, kind="ExternalInput").ap()
    wu0_d = nc.dram_tensor("Wu0", [F, H], F16, kind="ExternalInput").ap()
    b0_d = nc.dram_tensor("b0m", [128, KT], F32, kind="ExternalInput").ap()
    b1_d = nc.dram_tensor("b1m", [128, KT], F32, kind="ExternalInput").ap()
    fc_d = nc.dram_tensor("fc_w", [H, P], F16, kind="ExternalInput").ap()
    fcb_d = nc.dram_tensor("fc_bm", [P, 1], F32, kind="ExternalInput").ap()
    out_d = nc.dram_tensor("outT", [P, BL], F32, kind="ExternalOutput").ap()

    from contextlib import ExitStack

    with tile.TileContext(nc) as tc, ExitStack() as ctx:
        const = ctx.enter_context(tc.tile_pool(name="const", bufs=1))
        tanh_pool = ctx.enter_context(tc.tile_pool(name="tanh", bufs=4))
        psum = ctx.enter_context(tc.tile_pool(name="psum", bufs=8, space="PSUM"))

        def load_w8(nm):
            # 3D [128, KT, H] fp8 tile; sub-tile k holds W[k*128:(k+1)*128, :]
            t_ = const.tile([128, KT, H], F8, tag=nm)
            for k in range(KT):
                nc.sync.dma_start(t_[:, k : k + 1, :], w_d[nm][k * 128 : (k + 1) * 128, :])
            return t_

        # ---- load weights / constants, in the order t=0 consumes them -----
        xt = []
        for c in range(NXT):
            t_ = const.tile([128, BL], F16, tag=f"xt_{c}")
            xt.append(t_)
        nc.sync.dma_start(xt[0][:], xT_d[0:128, :])

        # 16 padded Wu0 weight tiles (fp16, pre-scaled by WS host-side)
        wu0 = []
        for j in range(TPX):
            t_ = const.tile([128, H], F16, tag=f"wu0_{j}")
            nc.gpsimd.memset(t_[:], 0.0)
            wu0.append(t_)
        nc.sync.dma_start(wu0[0][0:F, :], wu0_d[:, :])
        b0m = const.tile([128, KT], F32, tag="b0m")
        nc.sync.dma_start(b0m[:], b0_d[:, :])

        wu1h = load_w8("Wu1h")
        wu1l = load_w8("Wu1l")
        b1m = const.tile([128, KT], F32, tag="b1m")
        nc.sync.dma_start(b1m[:], b1_d[:, :])

        wh0h = load_w8("Wh0h")
        wh0l = load_w8("Wh0l")
        wh1h = load_w8("Wh1h")
        wh1l = load_w8("Wh1l")

        for j in range(1, TPX):
            nc.sync.dma_start(wu0[j][F * j : F * j + F, :], wu0_d[:, :])
        for c in range(1, NXT):
            nc.sync.dma_start(xt[c][:], xT_d[c * 128 : (c + 1) * 128, :])

        fcw = []
        for k in range(KT):
            t_ = const.tile([128, P], F16, tag=f"fcw_{k}")
            nc.sync.dma_start(t_[:], fc_d[k * 128 : (k + 1) * 128, :])
            fcw.append(t_)
        fcb = const.tile([P, 1], F32, tag="fcb")
        nc.sync.dma_start(fcb[:], fcb_d[:, :])

        # ---- state: fp16 master tiles + per-half fp8 matmul-input copies --
        g0 = [[None] * NH for _ in range(KT)]
        g1 = [[None] * NH for _ in range(KT)]
        for k in range(KT):
            for h in range(NH):
                a = const.tile([128, NHW], F16, tag=f"g0_{k}_{h}")
                g0[k][h] = a
                a = const.tile([128, NHW], F16, tag=f"g1_{k}_{h}")
                g1[k][h] = a
        q0 = []
        q1 = []
        for h in range(NH):
            a = const.tile([128, KT, NHW], F8, tag=f"q0_{h}")
            q0.append(a)
            a = const.tile([128, KT, NHW], F8, tag=f"q1_{h}")
            q1.append(a)

        outT = const.tile([P, BL], F32, tag="outT")

        def dr_chain(pz, whi, wlo, q, ms, start, stop=False):
            first = start
            for wi, wt in enumerate((whi, wlo)):
                for p in range(NP_):
                    last = wi == 1 and p == NP_ - 1
                    nc.tensor.matmul(
                        pz[:],
                        wt[:, 2 * p : 2 * p + 2, ms],
                        q[:, 2 * p : 2 * p + 2, :],
                        start=first,
                        stop=(stop and last),
                        perf_mode=DR,
                    )
                    first = False

        # ---- recurrence ----------------------------------------------------
        reps = int(os.environ.get("KERNEL_REPEAT", "1"))
        steps = [(t, t == 0 and r == 0) for r in range(reps) for t in range(T)]
        for t, first in steps:
            s_act = float(DEC**t / WS)
            c_upd = float(DT * DEC ** -(t + 1))
            xc, xj = t // TPX, t % TPX
            for h in range(NH):
                # cell 0: z0 = Wh0^T g0 + Wu0p^T x~_t (all scaled by WS)
                t0s = []
                for m in range(KT):
                    ms = slice(m * 128, (m + 1) * 128)
                    pz = psum.tile([128, NHW], F32, tag="pz")
                    if not first:
                        dr_chain(pz, wh0h, wh0l, q0[h], ms, start=True)
                    nc.tensor.matmul(
                        pz[:],
                        wu0[xj][:, ms],
                        xt[xc][:, h * NHW : (h + 1) * NHW],
                        start=first,
                        stop=True,
                    )
                    t0 = tanh_pool.tile([128, NHW], F16, tag="t0")
                    nc.scalar.activation(
                        t0[:], pz[:], AF.Tanh, bias=b0m[:, m : m + 1], scale=s_act
                    )
                    t0s.append(t0)
                for m in range(KT):
                    # fp16 master update + fp8 matmul-input copy
                    if first:
                        nc.vector.tensor_scalar_mul(g0[m][h][:], t0s[m][:], c_upd)
                    else:
                        nc.vector.scalar_tensor_tensor(
                            g0[m][h][:],
                            t0s[m][:],
                            c_upd,
                            g0[m][h][:],
                            ALU.mult,
                            ALU.add,
                        )
                    nc.vector.tensor_copy(q0[h][:, m : m + 1, :], g0[m][h][:])
                # cell 1: z1 = Wh1^T g1 + (0.9*Wu1)^T g0'
                t1s = []
                for m in range(KT):
                    ms = slice(m * 128, (m + 1) * 128)
                    pz = psum.tile([128, NHW], F32, tag="pz")
                    if not first:
                        dr_chain(pz, wh1h, wh1l, q1[h], ms, start=True)
                    dr_chain(pz, wu1h, wu1l, q0[h], ms, start=first, stop=True)
                    t1 = tanh_pool.tile([128, NHW], F16, tag="t1")
                    nc.scalar.activation(
                        t1[:], pz[:], AF.Tanh, bias=b1m[:, m : m + 1], scale=s_act
                    )
                    t1s.append(t1)
                for m in range(KT):
                    if first:
                        nc.vector.tensor_scalar_mul(g1[m][h][:], t1s[m][:], c_upd)
                    else:
                        nc.vector.scalar_tensor_tensor(
                            g1[m][h][:],
                            t1s[m][:],
                            c_upd,
                            g1[m][h][:],
                            ALU.mult,
                            ALU.add,
                        )
                    nc.gpsimd.tensor_copy(q1[h][:, m : m + 1, :], g1[m][h][:])

        # ---- output head: outT = 0.9^T * (fc_w^T g1) + fc_b ---------------
        for h in range(NH):
            po = psum.tile([128, NHW], F32, tag="pz")
            for k in range(KT):
                nc.tensor.matmul(
                    po[0:P, :],
                    fcw[k][:, 0:P],
                    g1[k][h][:],
                    start=(k == 0),
                    stop=(k == KT - 1),
                )
            nc.scalar.activation(
                outT[0:P, h * NHW : (h + 1) * NHW],
                po[0:P, :],
                AF.Identity,
                bias=fcb[:, 0:1],
                scale=float(DEC**T),
            )
            nc.sync.dma_start(
                out_d[:, h * NHW : (h + 1) * NHW],
                outT[0:P, h * NHW : (h + 1) * NHW],
            )

    nc.compile()
    return nc


_NC_CACHE = None


def _get_program():
    global _NC_CACHE
    if _NC_CACHE is None:
        _NC_CACHE = build_program()
    return _NC_CACHE


F8NP = mybir.dt.np(F8)


def _q8(a):
    return np.asarray(a, np.float32).astype(F8NP)


def _prep_inputs(x, Wh0, Wu0, b0, Wh1, Wu1, b1, fc_w, fc_b):
    """Host-side prep: shard/transpose/pad x; hi+lo fp8 weight split."""
    dec_inv = (DEC ** -np.arange(T, dtype=np.float64)).astype(np.float32)
    xs = (np.asarray(x[:, :T, :], np.float32) * dec_inv[None, :, None]).astype(
        np.float16
    )
    xp = np.zeros((XROWS, B), np.float16)
    xp[: T * F] = xs.transpose(1, 2, 0).reshape(T * F, B)

    def hilo(w):
        ws = np.asarray(w, np.float32) * np.float32(WS)
        hi = _q8(ws)
        lo = _q8(ws - hi.astype(np.float32))
        return hi, lo

    wh0h, wh0l = hilo(Wh0)
    wh1h, wh1l = hilo(Wh1)
    wu1h, wu1l = hilo(np.asarray(Wu1, np.float32) * np.float32(DEC))

    shared = {
        "Wh0h": wh0h,
        "Wh0l": wh0l,
        "Wh1h": wh1h,
        "Wh1l": wh1l,
        "Wu1h": wu1h,
        "Wu1l": wu1l,
        "Wu0": (np.asarray(Wu0, np.float32) * np.float32(WS)).astype(np.float16),
        "b0m": np.ascontiguousarray(np.asarray(b0, np.float32).reshape(KT, 128).T),
        "b1m": np.ascontiguousarray(np.asarray(b1, np.float32).reshape(KT, 128).T),
        "fc_w": np.asarray(fc_w, np.float32).astype(np.float16),
        "fc_bm": np.ascontiguousarray(np.asarray(fc_b, np.float32).reshape(P, 1)),
    }
    in_maps = []
    for c in range(NCORES):
        m = dict(shared)
        m["xTp"] = np.ascontiguousarray(xp[:, c * BL : (c + 1) * BL])
        in_maps.append(m)
    return in_maps


def run(inputs, trace=False, **kw):
    nc = _get_program()
    in_maps = _prep_inputs(**inputs)
    res = bass_utils.run_bass_kernel_spmd(
        nc, in_maps, core_ids=list(range(NCORES)), trace=trace, **kw
    )
    out = np.empty((B, P), np.float32)
    for c in range(NCORES):
        out[c * BL : (c + 1) * BL, :] = res.results[c]["outT"].T
    return out, res


def kernel(**inputs):
    out, _ = run(inputs, trace=False)
    return out


if __name__ == "__main__":
    print("smoke test: building program...")
    nc = _get_program()
    print("built ok")


# revision 17
# speedup vs baseline: 1.1851x; 1.1851x over previous
"""Trainium2 Bass kernel for the 2-layer liquid-NN multistep recurrence.

Math (reference):
    for t in 0..49:
        h0 = 0.9*h0 + 0.1*tanh(h0 @ Wh0 + x_t @ Wu0 + b0)
        h1 = 0.9*h1 + 0.1*tanh(h1 @ Wh1 + h0 @ Wu1 + b1)
    out = h1 @ fc_w + fc_b

Kernel strategy:
  - Data parallel over 8 NeuronCores: batch 8192 -> 1024 rows/core.
  - State kept TRANSPOSED in SBUF: tiles are [128(h), 512(b)], so every
    matmul contracts over the partition dim with naturally-laid-out weights
    (lhsT = W[h, ho] slice, rhs = state tile).
  - Rescaled state g_t = h_t / 0.9^t turns the update into a single fused
    axpy per tile:  g += (0.1*0.9^-(t+1)) * tanh(0.9^t * psum + b)
    (tanh input scale+bias ride free on the ACT instruction; the axpy is one
    DVE scalar_tensor_tensor).  Wu1 is pre-scaled by 0.9 host-side so both
    accumulation terms of cell 1 share the 0.9^t scale.
  - Everything the PE touches is fp16: same 11-bit mantissa as fp32r, but
    half the weight-load (LDWEIGHTS) traffic so back-to-back matmuls run at
    the 512-cycle floor.  PSUM accumulation stays fp32; the DVE axpy
    reads the f32 tanh and updates the fp16 state in place.
  - x ships compact as [8t+f, b] fp16 (rows past 8T zero-padded host-side),
    DMA'd as four dense [128, b] tiles = 16 timesteps each.  The x_t @ Wu0
    term is a full K=128 matmul against one of 16 on-device-built weight
    tiles (Wu0 at rows 8j..8j+8, zeros elsewhere) -- narrow-K matmuls run
    at half rate on the PE, full-K ones at line rate.
  - t=0 is specialized (state starts at zero): the Wh*state matmuls are
    skipped and the state is written directly, so no state memsets and the
    first matmul issues as soon as the first x tile lands.
"""

import os
import sys

import numpy as np

for _p in ("/opt/trn_rl_repo",):
    if _p not in sys.path:
        sys.path.insert(0, _p)

import concourse.bass as bass
import concourse.tile as tile
from concourse import bacc, bass_utils, mybir

F32 = mybir.dt.float32
F16 = mybir.dt.float16
AF = mybir.ActivationFunctionType
ALU = mybir.AluOpType

NCORES = 8
B = 8192
BL = B // NCORES  # 1024
S = 50
F = 8
H = 512
P = 10
T = 50
DT = 0.1
DEC = 1.0 - DT
KT = H // 128  # 4 k/ho tiles
NH = 2  # batch halves of 512
NHW = BL // NH  # 512

TPX = 128 // F  # 16 timesteps per x tile
NXT = (T + TPX - 1) // TPX  # 4 x tiles
XROWS = NXT * 128  # 512 padded x rows


def build_program():
    nc = bacc.Bacc(
        "TRN2", target_bir_lowering=False, debug=False, num_devices=NCORES
    )
    xT_d = nc.dram_tensor("xTp", [XROWS, BL], F16, kind="ExternalInput").ap()
    wh0_d = nc.dram_tensor("Wh0", [H, H], F16, kind="ExternalInput").ap()
    wh1_d = nc.dram_tensor("Wh1", [H, H], F16, kind="ExternalInput").ap()
    wu1_d = nc.dram_tensor("Wu1s", [H, H], F16, kind="ExternalInput").ap()
    wu0_d = nc.dram_tensor("Wu0", [F, H], F16, kind="ExternalInput").ap()
    b0_d = nc.dram_tensor("b0m", [128, KT], F32, kind="ExternalInput").ap()
    b1_d = nc.dram_tensor("b1m", [128, KT], F32, kind="ExternalInput").ap()
    fc_d = nc.dram_tensor("fc_w", [H, P], F16, kind="ExternalInput").ap()
    fcb_d = nc.dram_tensor("fc_bm", [P, 1], F32, kind="ExternalInput").ap()
    out_d = nc.dram_tensor("outT", [P, BL], F32, kind="ExternalOutput").ap()

    from contextlib import ExitStack

    with tile.TileContext(nc) as tc, ExitStack() as ctx:
        const = ctx.enter_context(tc.tile_pool(name="const", bufs=1))
        tanh_pool = ctx.enter_context(tc.tile_pool(name="tanh", bufs=4))
        psum = ctx.enter_context(tc.tile_pool(name="psum", bufs=8, space="PSUM"))

        # ---- load weights / constants, in the order t=0 consumes them -----
        xt = []
        for c in range(NXT):
            t_ = const.tile([128, BL], F16, tag=f"xt_{c}")
            xt.append(t_)
        nc.sync.dma_start(xt[0][:], xT_d[0:128, :])

        # 16 padded Wu0 weight tiles: Wu0 at rows 8j..8j+8, zeros elsewhere.
        # Built on-device: GpSimd memset + an 8-row DMA from the tiny Wu0.
        # Only j=0 is DMA'd up front -- j=1..15 queue after the big weight
        # tiles so they don't delay the t=0/t=1 critical DMAs.
        wu0 = []
        for j in range(TPX):
            t_ = const.tile([128, H], F16, tag=f"wu0_{j}")
            nc.gpsimd.memset(t_[:], 0.0)
            wu0.append(t_)
        nc.sync.dma_start(wu0[0][0:F, :], wu0_d[:, :])
        b0m = const.tile([128, KT], F32, tag="b0m")
        nc.sync.dma_start(b0m[:], b0_d[:, :])

        wu1 = []
        for k in range(KT):
            t_ = const.tile([128, H], F16, tag=f"wu1_{k}")
            nc.sync.dma_start(t_[:], wu1_d[k * 128 : (k + 1) * 128, :])
            wu1.append(t_)
        b1m = const.tile([128, KT], F32, tag="b1m")
        nc.sync.dma_start(b1m[:], b1_d[:, :])

        wh0 = []
        wh1 = []
        for k in range(KT):
            t_ = const.tile([128, H], F16, tag=f"wh0_{k}")
            nc.sync.dma_start(t_[:], wh0_d[k * 128 : (k + 1) * 128, :])
            wh0.append(t_)
        for k in range(KT):
            t_ = const.tile([128, H], F16, tag=f"wh1_{k}")
            nc.sync.dma_start(t_[:], wh1_d[k * 128 : (k + 1) * 128, :])
            wh1.append(t_)

        for j in range(1, TPX):
            nc.sync.dma_start(wu0[j][F * j : F * j + F, :], wu0_d[:, :])
        for c in range(1, NXT):
            nc.sync.dma_start(xt[c][:], xT_d[c * 128 : (c + 1) * 128, :])

        fcw = []
        for k in range(KT):
            t_ = const.tile([128, P], F16, tag=f"fcw_{k}")
            nc.sync.dma_start(t_[:], fc_d[k * 128 : (k + 1) * 128, :])
            fcw.append(t_)
        fcb = const.tile([P, 1], F32, tag="fcb")
        nc.sync.dma_start(fcb[:], fcb_d[:, :])

        # ---- state tiles (separate tile per k-block per half: avoids false
        # cross-half dependencies).  No memsets: the specialized t=0 step
        # writes them before anything reads them. ---------------------------
        g0 = [[None] * NH for _ in range(KT)]
        g1 = [[None] * NH for _ in range(KT)]
        for k in range(KT):
            for h in range(NH):
                a = const.tile([128, NHW], F16, tag=f"g0_{k}_{h}")
                g0[k][h] = a
                a = const.tile([128, NHW], F16, tag=f"g1_{k}_{h}")
                g1[k][h] = a

        outT = const.tile([P, BL], F32, tag="outT")

        # ---- recurrence ----------------------------------------------------
        reps = int(os.environ.get("KERNEL_REPEAT", "1"))
        steps = [(t, t == 0 and r == 0) for r in range(reps) for t in range(T)]
        for t, first in steps:
            s_in = float(DEC**t)
            c_upd = float(DT * DEC ** -(t + 1))
            xc, xj = t // TPX, t % TPX
            for h in range(NH):
                # cell 0: z0 = Wh0^T g0 + Wu0p^T x~_t.  Phase A: all matmul
                # groups + tanh against the OLD state; phase B: all updates.
                t0s = []
                for m in range(KT):
                    ms = slice(m * 128, (m + 1) * 128)
                    pz = psum.tile([128, NHW], F32, tag="pz")
                    if not first:
                        for k in range(KT):
                            nc.tensor.matmul(
                                pz[:],
                                wh0[k][:, ms],
                                g0[k][h][:],
                                start=(k == 0),
                                stop=False,
                            )
                    nc.tensor.matmul(
                        pz[:],
                        wu0[xj][:, ms],
                        xt[xc][:, h * NHW : (h + 1) * NHW],
                        start=first,
                        stop=True,
                    )
                    t0 = tanh_pool.tile([128, NHW], F32, tag="t0")
                    nc.scalar.activation(
                        t0[:], pz[:], AF.Tanh, bias=b0m[:, m : m + 1], scale=s_in
                    )
                    t0s.append(t0)
                for m in range(KT):
                    # g0[m] += c_upd * t0   (fused axpy)
                    if first:
                        nc.vector.tensor_scalar_mul(g0[m][h][:], t0s[m][:], c_upd)
                    else:
                        nc.vector.scalar_tensor_tensor(
                            g0[m][h][:],
                            t0s[m][:],
                            c_upd,
                            g0[m][h][:],
                            ALU.mult,
                            ALU.add,
                        )
                # cell 1: z1 = Wh1^T g1 + (0.9*Wu1)^T g0'
                t1s = []
                for m in range(KT):
                    ms = slice(m * 128, (m + 1) * 128)
                    pz = psum.tile([128, NHW], F32, tag="pz")
                    if not first:
                        for k in range(KT):
                            nc.tensor.matmul(
                                pz[:],
                                wh1[k][:, ms],
                                g1[k][h][:],
                                start=(k == 0),
                                stop=False,
                            )
                    for k in range(KT):
                        nc.tensor.matmul(
                            pz[:],
                            wu1[k][:, ms],
                            g0[k][h][:],
                            start=(first and k == 0),
                            stop=(k == KT - 1),
                        )
                    t1 = tanh_pool.tile([128, NHW], F32, tag="t1")
                    nc.scalar.activation(
                        t1[:], pz[:], AF.Tanh, bias=b1m[:, m : m + 1], scale=s_in
                    )
                    t1s.append(t1)
                for m in range(KT):
                    if first:
                        nc.vector.tensor_scalar_mul(g1[m][h][:], t1s[m][:], c_upd)
                    else:
                        nc.vector.scalar_tensor_tensor(
                            g1[m][h][:],
                            t1s[m][:],
                            c_upd,
                            g1[m][h][:],
                            ALU.mult,
                            ALU.add,
                        )

        # ---- output head: outT = 0.9^T * (fc_w^T g1) + fc_b ---------------
        for h in range(NH):
            po = psum.tile([128, NHW], F32, tag="pz")
            for k in range(KT):
                nc.tensor.matmul(
                    po[0:P, :],
                    fcw[k][:, 0:P],
                    g1[k][h][:],
                    start=(k == 0),
                    stop=(k == KT - 1),
                )
            nc.scalar.activation(
                outT[0:P, h * NHW : (h + 1) * NHW],
                po[0:P, :],
                AF.Identity,
                bias=fcb[:, 0:1],
                scale=float(DEC**T),
            )
            nc.sync.dma_start(
                out_d[:, h * NHW : (h + 1) * NHW],
                outT[0:P, h * NHW : (h + 1) * NHW],
            )

    nc.compile()
    return nc


_NC_CACHE = None


def _get_program():
    global _NC_CACHE
    if _NC_CACHE is None:
        _NC_CACHE = build_program()
    return _NC_CACHE


def _prep_inputs(x, Wh0, Wu0, b0, Wh1, Wu1, b1, fc_w, fc_b):
    """Host-side prep: shard + transpose/rescale/pad x, pre-scale Wu1."""
    dec_inv = (DEC ** -np.arange(T, dtype=np.float64)).astype(np.float32)
    # [B, S, F] -> take T steps, scale by 0.9^-t, -> [T, F, B] fp16,
    # flattened to rows 8t+f and zero-padded to XROWS.
    xs = (np.asarray(x[:, :T, :], np.float32) * dec_inv[None, :, None]).astype(
        np.float16
    )
    xp = np.zeros((XROWS, B), np.float16)
    xp[: T * F] = xs.transpose(1, 2, 0).reshape(T * F, B)

    shared = {
        "Wh0": np.asarray(Wh0, np.float32).astype(np.float16),
        "Wh1": np.asarray(Wh1, np.float32).astype(np.float16),
        "Wu1s": (np.asarray(Wu1, np.float32) * np.float32(DEC)).astype(np.float16),
        "Wu0": np.asarray(Wu0, np.float32).astype(np.float16),
        "b0m": np.ascontiguousarray(np.asarray(b0, np.float32).reshape(KT, 128).T),
        "b1m": np.ascontiguousarray(np.asarray(b1, np.float32).reshape(KT, 128).T),
        "fc_w": np.asarray(fc_w, np.float32).astype(np.float16),
        "fc_bm": np.ascontiguousarray(np.asarray(fc_b, np.float32).reshape(P, 1)),
    }
    in_maps = []
    for c in range(NCORES):
        m = dict(shared)
        m["xTp"] = np.ascontiguousarray(xp[:, c * BL : (c + 1) * BL])
        in_maps.append(m)
    return in_maps


def run(inputs, trace=False, **kw):
    nc = _get_program()
    in_maps = _prep_inputs(**inputs)
    res = bass_utils.run_bass_kernel_spmd(
        nc, in_maps, core_ids=list(range(NCORES)), trace=trace, **kw
    )
    out = np.empty((B, P), np.float32)
    for c in range(NCORES):
        out[c * BL : (c + 1) * BL, :] = res.results[c]["outT"].T
    return out, res


def kernel(**inputs):
    out, _ = run(inputs, trace=False)
    return out


if __name__ == "__main__":
    print("smoke test: building program...")
    nc = _get_program()
    print("built ok")
